# revision 10
# baseline (speedup 1.0000x reference)
"""Trainium2 Bass kernel for nn_BayesianMetaPosterior.

The reference loss algebraically reduces to

    loss = 100 * sum(metamean**2) + 0.5 * sum(log(fishers)) + C
    C    = D * (2*log(0.1) - 0.5*log(2*pi))

(the Mahalanobis term sum(fishers * (means - means)^2) is identically zero,
so `means` never needs to be read). The kernel shards the two reductions
across 8 NeuronCores data-parallel: each core DMAs its slice HBM->SBUF in
large contiguous tiles and the scalar engine computes ln() / square() with
the fused per-partition accumulate (accum_out), so each 5.35 MB tile costs
exactly one ACTIVATE. Per-tile partial sums [128, 8] are DMA'd back and the
final (tiny) reduction and affine happen on host in float64.

Written in raw Bass (explicit engine blocks + semaphores) because the axon
bass2jax->neuronx-cc codegen (a) allows at most ONE sync wait per
instruction, so all waits are standalone sequencer instructions, and
(b) rejects bass_isa raw-struct ops like tensor_tensor_reduce, so the
square also runs on ACT.
"""

import math
import sys
from contextlib import ExitStack

import numpy as np

sys.path.insert(0, "/opt/trn_rl_repo")

import concourse.bass as bass
import concourse.mybir as mybir
from concourse.bass_utils import run_bass_kernel_spmd

D = 21_389_512
M = 3
PRIOR_SIGMA = 0.1
N_CORES = 8
P = 128
FD = 10_446  # free-dim per tile; [128, 10446] f32 = 5.35 MB per DMA
BUFS = 4

MM_PER_CORE = D // N_CORES  # 2,673,689
FISH_PER_CORE = (M * D) // N_CORES  # 8,021,067
N_FTILES = 6  # 6*128*10446 = 8,022,528 >= FISH_PER_CORE
N_MTILES = 2  # 2*128*10446 = 2,674,176 >= MM_PER_CORE
N_TILES = N_FTILES + N_MTILES
assert N_FTILES * P * FD >= FISH_PER_CORE
assert N_MTILES * P * FD >= MM_PER_CORE

_CACHE = {}


def _build_nc():
    f32 = mybir.dt.float32
    nc = bass.Bass()
    fish = nc.declare_dram_parameter("fish", [N_FTILES, P, FD], f32, isOutput=False)
    mm = nc.declare_dram_parameter("mm", [N_MTILES, P, FD], f32, isOutput=False)
    acc_out = nc.declare_dram_parameter("acc", [P, N_TILES], f32, isOutput=True)

    with ExitStack() as ctx:
        slots = [
            ctx.enter_context(nc.sbuf_tensor(f"slot{i}", [P, FD], f32))
            for i in range(BUFS)
        ]
        acc = ctx.enter_context(nc.sbuf_tensor([P, N_TILES], f32))
        dum = ctx.enter_context(nc.sbuf_tensor([P, N_TILES], f32))
        # One semaphore per DMA: a single InstDMACopy is split across the 16
        # SDMA engines (16 independent +1 incs), so two DMAs sharing a sem
        # interleave and ">= 16" would not mean the first DMA finished.
        dsem = [
            ctx.enter_context(nc.semaphore(f"dsem{k}")) for k in range(N_TILES)
        ]
        osem = ctx.enter_context(nc.semaphore("osem"))
        act_sem = ctx.enter_context(nc.semaphore("act_sem"))
        block = ctx.enter_context(nc.Block())

        @block.sync
        def _(sync):
            for k in range(N_TILES):
                if k >= BUFS:
                    # wait for the consumer of slot (k-BUFS) before reuse
                    sync.wait_ge(act_sem, k - BUFS + 1)
                src = fish[k] if k < N_FTILES else mm[k - N_FTILES]
                sync.dma_start(out=slots[k % BUFS][:], in_=src).then_inc(dsem[k], 16)
            sync.wait_ge(act_sem, N_TILES)
            sync.dma_start(out=acc_out[:], in_=acc[:]).then_inc(osem, 16)
            sync.wait_ge(osem, 16)

        @block.scalar
        def _(scalar):
            for k in range(N_TILES):
                scalar.wait_ge(dsem[k], 16)
                func = (
                    mybir.ActivationFunctionType.Ln
                    if k < N_FTILES
                    else mybir.ActivationFunctionType.Square
                )
                nc.scalar.activation(
                    out=dum[:, k : k + 1].broadcast_to((P, FD)),
                    in_=slots[k % BUFS][:],
                    func=func,
                    accum_out=acc[:, k : k + 1],
                ).then_inc(act_sem, 1)

    nc.finalize()
    return nc


def _get_nc():
    if "nc" not in _CACHE:
        _CACHE["nc"] = _build_nc()
    return _CACHE["nc"]


def _in_maps(metamean, fishers):
    mm_flat = np.ascontiguousarray(metamean, dtype=np.float32).reshape(-1)
    fish_flat = np.ascontiguousarray(fishers, dtype=np.float32).reshape(-1)
    maps = []
    for c in range(N_CORES):
        fb = np.ones(N_FTILES * P * FD, dtype=np.float32)  # ln(1) = 0 padding
        fb[:FISH_PER_CORE] = fish_flat[c * FISH_PER_CORE : (c + 1) * FISH_PER_CORE]
        mb = np.zeros(N_MTILES * P * FD, dtype=np.float32)  # 0^2 = 0 padding
        mb[:MM_PER_CORE] = mm_flat[c * MM_PER_CORE : (c + 1) * MM_PER_CORE]
        maps.append(
            {
                "fish": fb.reshape(N_FTILES, P, FD),
                "mm": mb.reshape(N_MTILES, P, FD),
            }
        )
    return maps


def kernel(metamean, means, fishers, _trace=False):
    nc = _get_nc()
    res = run_bass_kernel_spmd(
        nc, _in_maps(metamean, fishers), core_ids=list(range(N_CORES)), trace=_trace
    )
    s_ln = 0.0
    s_sq = 0.0
    for r in res.results:
        a = r["acc"].astype(np.float64)
        s_ln += float(a[:, :N_FTILES].sum())
        s_sq += float(a[:, N_FTILES:].sum())
    const = D * (2.0 * math.log(PRIOR_SIGMA) - 0.5 * math.log(2.0 * math.pi))
    loss = 100.0 * s_sq + 0.5 * s_ln + const
    if _trace:
        kernel.last_exec_time_ns = res.exec_time_ns
    return np.asarray(loss, dtype=np.float32)


# revision 12
# speedup vs baseline: 1.1073x; 1.1073x over previous
"""Trainium2 Bass kernel for nn_BayesianMetaPosterior.

The reference loss algebraically reduces to

    loss = 100 * sum(metamean**2) + 0.5 * sum(log(fishers)) + C
    C    = D * (2*log(0.1) - 0.5*log(2*pi))

(the Mahalanobis term sum(fishers * (means - means)^2) is identically zero,
so `means` never needs to be read). The kernel shards the two reductions
across 8 NeuronCores data-parallel: each core DMAs its slice HBM->SBUF in
large contiguous tiles and the scalar engine computes ln() / square() with
the fused per-partition accumulate (accum_out), so each tile costs exactly
one ACTIVATE. Per-tile partial sums [128, 8] are DMA'd back and the final
(tiny) reduction and affine happen on host in float64.

Tile sizes shrink toward the end of the stream so every ACTIVATE fits
inside the remaining DMA time (ACT runs at ~0.57x the DMA byte rate); the
last tile is small, so the post-DMA tail is ~2us instead of a full 9us
ACTIVATE.

Written in raw Bass (explicit engine blocks + semaphores) because the axon
bass2jax->neuronx-cc codegen (a) allows at most ONE sync wait per
instruction, so all waits are standalone sequencer instructions, and
(b) rejects bass_isa raw-struct ops like tensor_tensor_reduce, so the
square also runs on ACT (Square shares the loaded table set with Ln).
"""

import math
import sys
from contextlib import ExitStack

import numpy as np

sys.path.insert(0, "/opt/trn_rl_repo")

import concourse.bass as bass
import concourse.mybir as mybir
from concourse.bass_utils import run_bass_kernel_spmd

D = 21_389_512
M = 3
PRIOR_SIGMA = 0.1
N_CORES = 8
P = 128

MM_PER_CORE = D // N_CORES  # 2,673,689
FISH_PER_CORE = (M * D) // N_CORES  # 8,021,067
FISH_FD = 62_666  # ceil(FISH_PER_CORE / 128), even; pad 181 elements of 1.0
MM_FD = 20_890  # ceil(MM_PER_CORE / 128), even; pad 231 elements of 0.0

# Stream order: (kind, free-dim). Sizes taper at the end so each ACT hides
# under the remaining DMA stream and the final tail is short.
TILES = [
    ("f", 18_872),
    ("f", 18_872),
    ("f", 18_870),
    ("m", 16_608),
    ("f", 6_052),
    ("m", 2_348),
    ("m", 910),
    ("m", 1_024),
]
assert sum(fd for k, fd in TILES if k == "f") == FISH_FD
assert sum(fd for k, fd in TILES if k == "m") == MM_FD
N_TILES = len(TILES)
MAX_FD = max(fd for _, fd in TILES)
BUFS = 2

_CACHE = {}


def _build_nc():
    f32 = mybir.dt.float32
    nc = bass.Bass()
    fish = nc.declare_dram_parameter("fish", [FISH_FD * P], f32, isOutput=False)
    mm = nc.declare_dram_parameter("mm", [MM_FD * P], f32, isOutput=False)
    acc_out = nc.declare_dram_parameter("acc", [P, N_TILES], f32, isOutput=True)

    with ExitStack() as ctx:
        slots = [
            ctx.enter_context(nc.sbuf_tensor(f"slot{i}", [P, MAX_FD], f32))
            for i in range(BUFS)
        ]
        acc = ctx.enter_context(nc.sbuf_tensor([P, N_TILES], f32))
        dum = ctx.enter_context(nc.sbuf_tensor([P, N_TILES], f32))
        # One semaphore per DMA: a single InstDMACopy is split across the 16
        # SDMA engines (16 independent +1 incs), so two DMAs sharing a sem
        # interleave and ">= 16" would not mean the first DMA finished.
        dsem = [
            ctx.enter_context(nc.semaphore(f"dsem{k}")) for k in range(N_TILES)
        ]
        osem = ctx.enter_context(nc.semaphore("osem"))
        act_sem = ctx.enter_context(nc.semaphore("act_sem"))
        block = ctx.enter_context(nc.Block())

        # per-tile source APs: contiguous [128, fd] views of the flat inputs
        srcs = []
        offs = {"f": 0, "m": 0}
        for kind, fd in TILES:
            base = fish if kind == "f" else mm
            o = offs[kind]
            srcs.append(base[o * P : (o + fd) * P].rearrange("(p f) -> p f", f=fd))
            offs[kind] = o + fd

        @block.sync
        def _(sync):
            for k, (kind, fd) in enumerate(TILES):
                if k >= BUFS:
                    # wait for the consumer of slot (k-BUFS) before reuse
                    sync.wait_ge(act_sem, k - BUFS + 1)
                sync.dma_start(
                    out=slots[k % BUFS][:, :fd], in_=srcs[k]
                ).then_inc(dsem[k], 16)
            sync.wait_ge(osem, 16)

        @block.scalar
        def _(scalar):
            for k, (kind, fd) in enumerate(TILES):
                scalar.wait_ge(dsem[k], 16)
                func = (
                    mybir.ActivationFunctionType.Ln
                    if kind == "f"
                    else mybir.ActivationFunctionType.Square
                )
                nc.scalar.activation(
                    out=dum[:, k : k + 1].broadcast_to((P, fd)),
                    in_=slots[k % BUFS][:, :fd],
                    func=func,
                    accum_out=acc[:, k : k + 1],
                ).then_inc(act_sem, 1)
            # ACT is an HWDGE engine: issue the (tiny) result DMA directly
            # from the ACT stream. The wait makes the last ACTIVATE's
            # accumulator write visible before the DMA engines read acc.
            scalar.wait_ge(act_sem, N_TILES)
            nc.scalar.dma_start(out=acc_out[:], in_=acc[:]).then_inc(osem, 16)

    nc.finalize()
    return nc


def _get_nc():
    if "nc" not in _CACHE:
        _CACHE["nc"] = _build_nc()
    return _CACHE["nc"]


def _in_maps(metamean, fishers):
    mm_flat = np.ascontiguousarray(metamean, dtype=np.float32).reshape(-1)
    fish_flat = np.ascontiguousarray(fishers, dtype=np.float32).reshape(-1)
    maps = []
    for c in range(N_CORES):
        fb = np.ones(FISH_FD * P, dtype=np.float32)  # ln(1) = 0 padding
        fb[:FISH_PER_CORE] = fish_flat[c * FISH_PER_CORE : (c + 1) * FISH_PER_CORE]
        mb = np.zeros(MM_FD * P, dtype=np.float32)  # 0^2 = 0 padding
        mb[:MM_PER_CORE] = mm_flat[c * MM_PER_CORE : (c + 1) * MM_PER_CORE]
        maps.append({"fish": fb, "mm": mb})
    return maps


def kernel(metamean, means, fishers, _trace=False):
    nc = _get_nc()
    res = run_bass_kernel_spmd(
        nc, _in_maps(metamean, fishers), core_ids=list(range(N_CORES)), trace=_trace
    )
    f_cols = [k for k, (kind, _) in enumerate(TILES) if kind == "f"]
    m_cols = [k for k, (kind, _) in enumerate(TILES) if kind == "m"]
    s_ln = 0.0
    s_sq = 0.0
    for r in res.results:
        a = r["acc"].astype(np.float64)
        s_ln += float(a[:, f_cols].sum())
        s_sq += float(a[:, m_cols].sum())
    const = D * (2.0 * math.log(PRIOR_SIGMA) - 0.5 * math.log(2.0 * math.pi))
    loss = 100.0 * s_sq + 0.5 * s_ln + const
    if _trace:
        kernel.last_exec_time_ns = res.exec_time_ns
    return np.asarray(loss, dtype=np.float32)
